# revision 25
# baseline (speedup 1.0000x reference)
"""GQA cross-attention block on 8 trn2 NeuronCores — phase-interleaved.

Sharding: tensor-parallel over heads. Core c owns KV group g=c (64 dims of
K/V) and its 4 query heads (256 q channels). Each core computes its heads'
attention plus its slice of the o-projection (rows c*256:(c+1)*256 of Wo),
producing a full-shape partial output; the host sums the 8 partials and
adds bo. No device collectives needed.

Structure: the attention inner loop is ACT-bound (exp of [128,1024]
scores ~1.1us per key-chunk vs ~0.85us of PE work), while the projection
and o-projection phases are PE-bound with ACT idle. The emission
interleaves them: b0's projection ramp carries the first attention unit
chunk-by-chunk; b1's projections and b0's o-proj quanta are pumped into
the attention kc-loop ticks (filler between the AV matmuls and the next
score pair, where the PE would otherwise wait on exp); b1's o-proj
quanta interleave as each query-chunk's normalize completes. All PSUM
evacuations + biases run on DVE (tensor_scalar_add with per-partition
bias), keeping ACT pure-exp. 1/Z is broadcast across the 64 head dims
via gpsimd.partition_broadcast instead of a PE ones-matmul.

PSUM budget (8 banks): 2x[128,1024] score slots (4) + 2 AV accumulators
(2) + 2 aux slots (2) shared by proj accums / o-proj / transposes.

Score matmuls: 64-contraction pairs on the two PE row-halves (kT
duplicated into both halves) run concurrently; two score pairs are
emitted back-to-back per tick to amortize the PE weight-buffer stall
of entering/leaving row-tiled pairs (the pair occupies both stationary
buffers, so the next LDWEIGHTS serializes). The softmax epilogue drains
each AV accumulator in one bf16 copy (Z rides as row 64) and defers the
reciprocal to kc==2 and the broadcast+normalize to kc==8 of the next
unit. DMA queues: enc/wq split across sync+gpsimd+scalar so the ramp is
never DMA-issue-paced; b1 x-tiles ride sync to keep gpsimd responsive
for the broadcasts.
"""

import numpy as np
import ml_dtypes

import concourse.bass as bass
from concourse import bacc
import concourse.mybir as mybir
import concourse.tile as tile
from concourse.bass_utils import run_bass_kernel_spmd
from concourse.masks import make_identity
from concourse.alu_op_type import AluOpType

BF16 = ml_dtypes.bfloat16
F32 = mybir.dt.float32
F16 = mybir.dt.float16
BF = mybir.dt.bfloat16

B = 2
S = 2048
HID = 2048
D = 64          # head dim
RQ = 4          # query heads per core (per kv group)
CH = RQ * D     # 256 q channels per core
NCORES = 8
NH = HID // 128  # 16 hidden chunks
NST = S // 512   # 4 s-tiles of 512
NKC = S // 128   # 16 key chunks of 128
SCALE = 1.0 / np.sqrt(D)


def _build_nc() -> bass.Bass:
    nc = bacc.Bacc()

    xT = nc.dram_tensor("xT", [B, HID, S], BF, kind="ExternalInput")
    encT = nc.dram_tensor("encT", [B, HID, S], BF, kind="ExternalInput")
    wq = nc.dram_tensor("wq", [HID, CH], BF, kind="ExternalInput")
    wkv = nc.dram_tensor("wkv", [HID, 2 * D], BF, kind="ExternalInput")
    wo = nc.dram_tensor("wo", [CH, HID], BF, kind="ExternalInput")
    bq = nc.dram_tensor("bq", [CH, 1], F32, kind="ExternalInput")
    bkk = nc.dram_tensor("bkk", [128, 1], F32, kind="ExternalInput")  # [bk;bk]
    bvv = nc.dram_tensor("bvv", [128, 1], F32, kind="ExternalInput")  # [bv;bv]
    out = nc.dram_tensor("out", [B, S, HID], BF, kind="ExternalOutput")

    ID = mybir.ActivationFunctionType.Identity
    EXP = mybir.ActivationFunctionType.Exp

    with tile.TileContext(nc) as tc:
        with (
            tc.tile_pool(name="wpool", bufs=1) as wpool,
            tc.tile_pool(name="xs", bufs=34) as xs_pool,
            tc.tile_pool(name="es", bufs=20) as es_pool,
            tc.tile_pool(name="acts", bufs=2) as acts,
            tc.tile_pool(name="vaug", bufs=2 * NKC) as vaug_pool,
            tc.tile_pool(name="epool", bufs=8) as epool,
            tc.tile_pool(name="small", bufs=4) as small,
            tc.tile_pool(name="osb", bufs=4) as osb_pool,
            tc.tile_pool(name="psb", bufs=2, space="PSUM") as ps_big,
            tc.tile_pool(name="psp", bufs=2, space="PSUM") as ps_pav,
            tc.tile_pool(name="psa", bufs=2, space="PSUM") as ps_aux,
        ):
            # ---- resident weights ----
            # wkv + biases on the scalar DMA queue (idle at start) so the
            # sync queue serves the first enc tiles immediately; wq/bq/wo
            # are emitted on sync after st0's enc DMAs (see ramp below).
            wq_t = []
            wkv_t = []
            for h in range(NH):
                wkvh = wpool.tile([128, 2 * D], BF, name=f"wkv{h}")
                nc.scalar.dma_start(out=wkvh[:], in_=wkv[h * 128:(h + 1) * 128, :])
                wkv_t.append(wkvh)
            bkk_t = wpool.tile([128, 1], F32, name="bkk_t")
            nc.scalar.dma_start(out=bkk_t[:], in_=bkk[:, :])
            bvv_t = wpool.tile([128, 1], F32, name="bvv_t")
            nc.scalar.dma_start(out=bvv_t[:], in_=bvv[:, :])
            bq_t = []
            wo_t = []

            def load_late_weights():
                # split wq across sync+gpsimd so b0's et st1+ DMAs aren't
                # stuck behind it; bq/wo ride the scalar queue's tail
                for h in range(NH):
                    wqh = wpool.tile([128, CH], BF, name=f"wq{h}")
                    eng = nc.sync if h % 2 == 0 else nc.gpsimd
                    eng.dma_start(out=wqh[:], in_=wq[h * 128:(h + 1) * 128, :])
                    wq_t.append(wqh)
                for cchunk in range(2):
                    bqc = wpool.tile([128, 1], F32, name=f"bq{cchunk}")
                    nc.scalar.dma_start(
                        out=bqc[:], in_=bq[cchunk * 128:(cchunk + 1) * 128, :])
                    bq_t.append(bqc)
                for cchunk in range(2):
                    woc = wpool.tile([128, HID], BF, name=f"wo{cchunk}")
                    nc.scalar.dma_start(
                        out=woc[:], in_=wo[cchunk * 128:(cchunk + 1) * 128, :])
                    wo_t.append(woc)

            ident = wpool.tile([128, 128], BF, name="ident")
            make_identity(nc, ident[:])

            # persistent v_aug tiles; ones column written once
            va_tiles = [
                [vaug_pool.tile([128, D + 1], BF, tag="vaug", name=f"va{b}_{kc}")
                 for kc in range(NKC)]
                for b in range(B)
            ]
            for b in range(B):
                for kc in range(NKC):
                    nc.gpsimd.memset(va_tiles[b][kc][:, D:D + 1], 1.0)

            # per-batch activation tiles (tags give b0/b1 separate slots)
            qp = {}
            kT2 = {}
            vT = {}
            oT = {}
            for b in range(B):
                qp[b] = [acts.tile([128, S], BF, tag=f"qp{i}", name=f"qp{i}_{b}")
                         for i in range(2)]
                kT2[b] = acts.tile([128, S], BF, tag="kT2", name=f"kT2{b}")
                vT[b] = acts.tile([D, S], BF, tag="vT", name=f"vT{b}")
                oT[b] = [acts.tile([128, S], BF, tag=f"oT{i}", name=f"oT{i}_{b}")
                         for i in range(2)]

            # ---- deferred softmax epilogue ----
            # (un, zs, row, qc, odst, b) -> recip at kc==2, bc+STT at kc==8
            pending = []
            pending_bc = []
            stt_done = {}      # (b, qc) -> count of normalized (rpair, half)
            on_qc_done = {}    # (b, qc) -> callback when all 4 STTs emitted

            def flush_recip():
                while pending:
                    un, zs, row, qc, odst, b = pending.pop(0)
                    key = f"{b}{row}{qc}{len(pending)}"
                    rt = small.tile([1, 512], F32, tag="rt", name=f"rt{key}")
                    nc.vector.reciprocal_approx_fast(rt[:], zs[:])
                    rth = small.tile([1, 512], BF, tag="rth", name=f"rth{key}")
                    nc.vector.tensor_copy(rth[:], rt[:])
                    pending_bc.append((un, rth, row, qc, odst, b))

            def flush_bc():
                while pending_bc:
                    un, rth, row, qc, odst, b = pending_bc.pop(0)
                    qsl = slice(qc * 512, (qc + 1) * 512)
                    key = f"{b}{row}{qc}{len(pending_bc)}"
                    # broadcast 1/Z across the 64 head dims on gpsimd (keeps
                    # the PE free of broadcast matmuls and their LDW stalls)
                    bcs = small.tile([D, 512], BF, tag="bcs", name=f"bcs{key}")
                    nc.gpsimd.partition_broadcast(bcs[:], rth[:], channels=D)
                    nc.vector.tensor_mul(odst[row:row + D, qsl], un[0:D, :], bcs[:])
                    k2 = (b, qc)
                    stt_done[k2] = stt_done.get(k2, 0) + 1
                    if stt_done[k2] == 4 and k2 in on_qc_done:
                        on_qc_done.pop(k2)()

            def flush_epilogue():
                flush_recip()
                flush_bc()

            # ---- work quanta ----
            filler = []        # list of 0-arg closures, FIFO

            def pump(n=1):
                for _ in range(n):
                    if filler:
                        filler.pop(0)()

            def pump_all():
                while filler:
                    filler.pop(0)()

            # projection quanta for one (b, st); act_bias=True routes the
            # PSUM evacuation + bias through ACT (ramp), else DVE.
            def proj_quanta(b, st):
                ssl = slice(st * 512, (st + 1) * 512)
                xt = [None] * NH
                et = [None] * NH
                state = {}

                def e_dma():
                    for h in range(NH):
                        e = es_pool.tile([128, 512], BF, tag="es",
                                         name=f"es{b}{st}{h}")
                        # ramp st0: split enc loads across two queues so the
                        # first kv matmuls aren't DMA-issue-paced
                        eng = (nc.gpsimd if (b == 0 and st == 0 and h % 2)
                               else nc.sync)
                        eng.dma_start(
                            out=e[:], in_=encT[b, h * 128:(h + 1) * 128, ssl])
                        et[h] = e

                def x_dma():
                    # b1 x-loads go on sync: the gpsimd queue must stay
                    # responsive for the 1/Z partition broadcasts
                    eng = nc.gpsimd if b == 0 else nc.sync
                    for h in range(NH):
                        x = xs_pool.tile([128, 512], BF, tag="xs",
                                         name=f"xs{b}{st}{h}")
                        eng.dma_start(
                            out=x[:], in_=xT[b, h * 128:(h + 1) * 128, ssl])
                        xt[h] = x

                def q_kv(i):
                    def f():
                        if i == 0:
                            state["kvp"] = ps_aux.tile(
                                [128, 512], F32, tag="aux", name=f"kvp{b}{st}")
                        kvp = state["kvp"]
                        for h in range(4 * i, 4 * i + 4):
                            nc.tensor.matmul(
                                kvp[:], wkv_t[h][:], et[h][:],
                                start=(h == 0), stop=(h == NH - 1))
                    return f

                def q_kv_evac():
                    kvp = state["kvp"]
                    nc.vector.tensor_scalar_add(
                        kT2[b][0:D, ssl], kvp[0:D, :], bkk_t[0:D, :])
                    nc.vector.tensor_scalar_add(
                        kT2[b][D:128, ssl], kvp[0:D, :], bkk_t[0:D, :])
                    nc.vector.tensor_scalar_add(
                        vT[b][:, ssl], kvp[D:128, :], bvv_t[D:128, :])

                def q_vaug():
                    for j in range(4):
                        kc = st * 4 + j
                        vtp = ps_aux.tile([128, D], BF, tag="aux",
                                          name=f"vtp{b}{kc}")
                        nc.tensor.transpose(
                            vtp[:], vT[b][:, kc * 128:(kc + 1) * 128],
                            ident[0:D, 0:D])
                        nc.vector.tensor_copy(va_tiles[b][kc][:, 0:D], vtp[:])

                def q_q(half, i):
                    def f():
                        key = f"qp{half}"
                        if i == 0:
                            state[key] = ps_aux.tile(
                                [128, 512], F32, tag="aux",
                                name=f"qps{b}{st}{half}")
                        qps = state[key]
                        csl = slice(half * 128, (half + 1) * 128)
                        for h in range(4 * i, 4 * i + 4):
                            nc.tensor.matmul(
                                qps[:], wq_t[h][:, csl], xt[h][:],
                                start=(h == 0), stop=(h == NH - 1))
                    return f

                def q_q_evac(half):
                    def f():
                        qps = state[f"qp{half}"]
                        nc.vector.tensor_scalar_add(
                            qp[b][half][:, ssl], qps[:], bq_t[half][:])
                    return f

                kv_chain = [e_dma] + [q_kv(i) for i in range(4)] + [q_kv_evac, q_vaug]
                qa_chain = [x_dma] + [q_q(0, i) for i in range(4)] + [q_q_evac(0)]
                qb_chain = [q_q(1, i) for i in range(4)] + [q_q_evac(1)]
                return kv_chain, qa_chain, qb_chain

            # o-projection quanta for (b, qc): 4 s-chunks x 4 hid quarters,
            # one aux slot per quantum so the slot turns over quickly
            def oproj_quanta(b, qc):
                quanta = []
                for si in range(4):
                    s128 = slice((qc * 4 + si) * 128, (qc * 4 + si + 1) * 128)
                    for hh in range(4):
                        def f(s128=s128, hh=hh, si=si):
                            ops = ps_aux.tile([128, 512], F32, tag="aux",
                                              name=f"op{b}{qc}{si}{hh}")
                            hsl = slice(hh * 512, (hh + 1) * 512)
                            nc.tensor.matmul(ops[:], oT[b][0][:, s128],
                                             wo_t[0][:, hsl], start=True, stop=False)
                            nc.tensor.matmul(ops[:], oT[b][1][:, s128],
                                             wo_t[1][:, hsl], start=False, stop=True)
                            osb = osb_pool.tile([128, 512], BF, tag="osb",
                                                name=f"ob{b}{qc}{si}{hh}")
                            nc.vector.tensor_copy(osb[:], ops[:])
                            nc.sync.dma_start(
                                out=out[b, s128, hsl], in_=osb[:])
                        quanta.append(f)
                return quanta

            # attention unit (b, rpair, qc); tick(kc) emits one kc step,
            # pump_rate fillers pulled per odd kc
            class Unit:
                def __init__(self, b, rpair, qc, pump_rate=1):
                    self.b, self.rpair, self.qc = b, rpair, qc
                    self.pump_rate = pump_rate
                    self.qpt = qp[b][rpair]
                    self.odst = oT[b][rpair]
                    self.qsl = slice(qc * 512, (qc + 1) * 512)
                    self.ava = ps_pav.tile([D + 1, 512], F32, tag="pav",
                                           name=f"av{b}{rpair}{qc}a")
                    self.avb = ps_pav.tile([D + 1, 512], F32, tag="pav",
                                           name=f"av{b}{rpair}{qc}b")
                    self.es = [None] * NKC

                def _score(self, kc):
                    b, rpair, qc = self.b, self.rpair, self.qc
                    ksl = slice(kc * 128, (kc + 1) * 128)
                    sct2 = ps_big.tile([128, 1024], F32, tag="big",
                                       name=f"sc{b}{rpair}{qc}{kc}")
                    nc.tensor.matmul(
                        sct2[:, 0:512], kT2[b][0:D, ksl],
                        self.qpt[0:D, self.qsl], start=True, stop=True)
                    nc.tensor.matmul(
                        sct2[:, 512:1024], kT2[b][D:128, ksl],
                        self.qpt[D:128, self.qsl], start=True, stop=True)
                    e2 = epool.tile([128, 1024], BF, tag="e",
                                    name=f"e{b}{rpair}{qc}{kc}")
                    nc.scalar.activation(e2[:], sct2[:], EXP, scale=float(SCALE))
                    self.es[kc] = e2

                def _av(self, kc, stop=False):
                    b = self.b
                    pe2 = self.es[kc]
                    nc.tensor.matmul(
                        self.ava[:], va_tiles[b][kc][:], pe2[:, 0:512],
                        start=(kc == 0), stop=stop)
                    nc.tensor.matmul(
                        self.avb[:], va_tiles[b][kc][:], pe2[:, 512:1024],
                        start=(kc == 0), stop=stop)

                def tick2(self, j):
                    # two score pairs back-to-back (amortizes the PE weight-
                    # buffer stall of entering/leaving row-tiled pairs), then
                    # the AV matmuls of the previous pair of kc's
                    self._score(2 * j)
                    self._score(2 * j + 1)
                    if j > 0:
                        self._av(2 * j - 2)
                        self._av(2 * j - 1)
                    if j == 1:
                        flush_recip()
                    elif j == 4:
                        flush_bc()
                    pump(self.pump_rate if j % 2 == 0 else 2 * self.pump_rate)

                def finish(self):
                    b, rpair, qc = self.b, self.rpair, self.qc
                    self._av(NKC - 2)
                    self._av(NKC - 1, stop=True)
                    # drain each accumulator in ONE copy (frees the PSUM bank
                    # fastest); Z rides along as row D, re-cast to f32 for
                    # the reciprocal later
                    for half, av in ((0, self.ava), (1, self.avb)):
                        key = f"{b}{rpair}{qc}{half}"
                        un = small.tile([D + 1, 512], BF, tag="un", bufs=6,
                                        name=f"un{key}")
                        nc.vector.tensor_copy(un[:], av[:])
                        zs = small.tile([1, 512], F32, tag="zs", name=f"zs{key}")
                        nc.vector.tensor_copy(zs[:], un[D:D + 1, :])
                        pending.append((un, zs, half * D, qc, self.odst, b))

            def emit_unit(b, rpair, qc, pump_rate=1):
                u = Unit(b, rpair, qc, pump_rate)
                for j in range(NKC // 2):
                    u.tick2(j)
                u.finish()

            # =========== emission schedule ===========
            # ramp: projection of b0 (ACT bias; ACT idle anyway) with the
            # first attention unit chunked in per-st so exp starts early
            u0 = None
            for st in range(NST):
                kv_c, qa_c, qb_c = proj_quanta(0, st)
                for f in kv_c:
                    f()
                if st == 0:
                    load_late_weights()
                    for f in qa_c + qb_c:
                        f()
                    u0 = Unit(0, 0, 0)
                    u0.tick2(0)
                else:
                    u0.tick2(2 * st - 1)
                    for f in qa_c:
                        f()
                    u0.tick2(2 * st)
                    for f in qb_c:
                        f()
            u0.tick2(NKC // 2 - 1)
            u0.finish()

            # window 1: rest of attn(b0); filler = proj(b1) then oproj(b0)
            b1_chains = [proj_quanta(1, st) for st in range(NST)]
            # kv of all st first (needed by every b1 unit), then qA st0
            for st in range(NST):
                filler.extend(b1_chains[st][0])
            filler.extend(b1_chains[0][1])           # qA st0

            def need_q(half, st):
                # force-drain filler up to and including qX(st) of b1;
                # queue order guarantees every prerequisite is ahead of it
                chain = b1_chains[st][1 + half]
                while chain[-1] in filler:
                    pump(1)

            for qc in range(4):
                on_qc_done[(0, qc)] = (
                    lambda qc=qc: filler.extend(oproj_quanta(0, qc)))
                if qc > 0:
                    emit_unit(0, 0, qc)
                emit_unit(0, 1, qc)
            # remaining b1 q-projections, ordered so each st's x-tiles die
            # (at qB) before the next st's load: qB0, qA1, qB1, qA2, ...
            filler.extend(b1_chains[0][2])
            for st in range(1, NST):
                filler.extend(b1_chains[st][1])
                filler.extend(b1_chains[st][2])

            # window 2: attn(b1) with filler = rest of b1 proj + oproj(b0/b1)
            for qc in range(4):
                on_qc_done[(1, qc)] = (
                    lambda qc=qc: filler.extend(oproj_quanta(1, qc)))
                need_q(0, qc)
                emit_unit(1, 0, qc)
                need_q(1, qc)
                # drain harder near the end so the tail is short
                emit_unit(1, 1, qc, pump_rate=1 if qc < 3 else 2)

            flush_epilogue()
            pump_all()

    if not nc.is_finalized():
        nc.finalize()
    return nc


_NC = None
_RUN_KWARGS = {}
_LAST_RESULT = None


def _get_nc():
    global _NC
    if _NC is None:
        _NC = _build_nc()
    return _NC


def kernel(x, encoder_output, Wq, bq, Wk, bk, Wv, bv, Wo, bo):
    nc = _get_nc()
    xT = np.ascontiguousarray(np.asarray(x, np.float32).transpose(0, 2, 1)).astype(BF16)
    encT = np.ascontiguousarray(
        np.asarray(encoder_output, np.float32).transpose(0, 2, 1)).astype(BF16)
    Wq = np.asarray(Wq, np.float32)
    Wk = np.asarray(Wk, np.float32)
    Wv = np.asarray(Wv, np.float32)
    Wo = np.asarray(Wo, np.float32)
    bk = np.asarray(bk, np.float32)
    bv = np.asarray(bv, np.float32)
    in_maps = []
    for c in range(NCORES):
        csl = slice(c * CH, (c + 1) * CH)
        gsl = slice(c * D, (c + 1) * D)
        bk_g = bk[gsl].reshape(D, 1)
        bv_g = bv[gsl].reshape(D, 1)
        in_maps.append({
            "xT": xT,
            "encT": encT,
            "wq": np.ascontiguousarray(Wq[:, csl]).astype(BF16),
            "wkv": np.ascontiguousarray(
                np.concatenate([Wk[:, gsl], Wv[:, gsl]], axis=1)).astype(BF16),
            "wo": np.ascontiguousarray(Wo[csl, :]).astype(BF16),
            "bq": np.ascontiguousarray(
                np.asarray(bq, np.float32)[csl].reshape(CH, 1)),
            "bkk": np.ascontiguousarray(np.concatenate([bk_g, bk_g], axis=0)),
            "bvv": np.ascontiguousarray(np.concatenate([bv_g, bv_g], axis=0)),
        })
    res = run_bass_kernel_spmd(nc, in_maps, list(range(NCORES)), **_RUN_KWARGS)
    global _LAST_RESULT
    _LAST_RESULT = res
    total = np.zeros((B, S, HID), np.float32)
    for c in range(NCORES):
        total += res.results[c]["out"].astype(np.float32)
    return total + np.asarray(bo, np.float32)


# revision 28
# speedup vs baseline: 1.0078x; 1.0078x over previous
"""GQA cross-attention block on 8 trn2 NeuronCores — phase-interleaved.

Sharding: tensor-parallel over heads. Core c owns KV group g=c (64 dims of
K/V) and its 4 query heads (256 q channels). Each core computes its heads'
attention plus its slice of the o-projection (rows c*256:(c+1)*256 of Wo),
producing a full-shape partial output; the host sums the 8 partials and
adds bo. No device collectives needed.

Structure: the attention inner loop is ACT-bound (exp of [128,1024]
scores ~1.1us per key-chunk vs ~0.85us of PE work), while the projection
and o-projection phases are PE-bound with ACT idle. The emission
interleaves them: b0's projection ramp carries the first attention unit
chunk-by-chunk; b1's projections and b0's o-proj quanta are pumped into
the attention kc-loop ticks (filler between the AV matmuls and the next
score pair, where the PE would otherwise wait on exp); b1's o-proj
quanta interleave as each query-chunk's normalize completes. All PSUM
evacuations + biases run on DVE (tensor_scalar_add with per-partition
bias), keeping ACT pure-exp. 1/Z is broadcast across the 64 head dims
via gpsimd.partition_broadcast instead of a PE ones-matmul.

PSUM budget (8 banks): 2x[128,1024] score slots (4) + 2 AV accumulators
(2) + 2 aux slots (2) shared by proj accums / o-proj / transposes.

Score matmuls: 64-contraction pairs on the two PE row-halves (kT
duplicated into both halves) run concurrently; two score pairs are
emitted back-to-back per tick to amortize the PE weight-buffer stall
of entering/leaving row-tiled pairs (the pair occupies both stationary
buffers, so the next LDWEIGHTS serializes). The softmax epilogue drains
each AV accumulator in one bf16 copy (Z rides as row 64) and defers the
reciprocal to kc==2 and the broadcast+normalize to kc==8 of the next
unit. DMA queues: enc/wq split across sync+gpsimd+scalar so the ramp is
never DMA-issue-paced; b1 x-tiles ride sync to keep gpsimd responsive
for the broadcasts.
"""

import numpy as np
import ml_dtypes

import concourse.bass as bass
from concourse import bacc
import concourse.mybir as mybir
import concourse.tile as tile
from concourse.bass_utils import run_bass_kernel_spmd
from concourse.masks import make_identity
from concourse.alu_op_type import AluOpType

BF16 = ml_dtypes.bfloat16
F32 = mybir.dt.float32
F16 = mybir.dt.float16
BF = mybir.dt.bfloat16

B = 2
S = 2048
HID = 2048
D = 64          # head dim
RQ = 4          # query heads per core (per kv group)
CH = RQ * D     # 256 q channels per core
NCORES = 8
NH = HID // 128  # 16 hidden chunks
NST = S // 512   # 4 s-tiles of 512
NKC = S // 128   # 16 key chunks of 128
SCALE = 1.0 / np.sqrt(D)


def _build_nc() -> bass.Bass:
    nc = bacc.Bacc()

    xT = nc.dram_tensor("xT", [B, HID, S], BF, kind="ExternalInput")
    encT = nc.dram_tensor("encT", [B, HID, S], BF, kind="ExternalInput")
    wq = nc.dram_tensor("wq", [HID, CH], BF, kind="ExternalInput")
    wkv = nc.dram_tensor("wkv", [HID, 2 * D], BF, kind="ExternalInput")
    wo = nc.dram_tensor("wo", [CH, HID], BF, kind="ExternalInput")
    bq = nc.dram_tensor("bq", [CH, 1], F32, kind="ExternalInput")
    bkk = nc.dram_tensor("bkk", [128, 1], F32, kind="ExternalInput")  # [bk;bk]
    bvv = nc.dram_tensor("bvv", [128, 1], F32, kind="ExternalInput")  # [bv;bv]
    out = nc.dram_tensor("out", [B, S, HID], BF, kind="ExternalOutput")

    ID = mybir.ActivationFunctionType.Identity
    EXP = mybir.ActivationFunctionType.Exp

    with tile.TileContext(nc) as tc:
        with (
            tc.tile_pool(name="wpool", bufs=1) as wpool,
            tc.tile_pool(name="xs", bufs=34) as xs_pool,
            tc.tile_pool(name="es", bufs=20) as es_pool,
            tc.tile_pool(name="acts", bufs=2) as acts,
            tc.tile_pool(name="vaug", bufs=2 * NKC) as vaug_pool,
            tc.tile_pool(name="epool", bufs=8) as epool,
            tc.tile_pool(name="small", bufs=4) as small,
            tc.tile_pool(name="osb", bufs=4) as osb_pool,
            tc.tile_pool(name="psb", bufs=2, space="PSUM") as ps_big,
            tc.tile_pool(name="psp", bufs=2, space="PSUM") as ps_pav,
            tc.tile_pool(name="psa", bufs=2, space="PSUM") as ps_aux,
        ):
            # ---- resident weights ----
            # wkv + biases on the scalar DMA queue (idle at start) so the
            # sync queue serves the first enc tiles immediately; wq/bq/wo
            # are emitted on sync after st0's enc DMAs (see ramp below).
            wq_t = []
            wkv_t = []
            for h in range(NH):
                wkvh = wpool.tile([128, 2 * D], BF, name=f"wkv{h}")
                nc.scalar.dma_start(out=wkvh[:], in_=wkv[h * 128:(h + 1) * 128, :])
                wkv_t.append(wkvh)
            bkk_t = wpool.tile([128, 1], F32, name="bkk_t")
            nc.scalar.dma_start(out=bkk_t[:], in_=bkk[:, :])
            bvv_t = wpool.tile([128, 1], F32, name="bvv_t")
            nc.scalar.dma_start(out=bvv_t[:], in_=bvv[:, :])
            bq_t = []
            wo_t = []

            def load_late_weights():
                # split wq across sync+gpsimd so b0's et st1+ DMAs aren't
                # stuck behind it; bq/wo ride the scalar queue's tail
                for h in range(NH):
                    wqh = wpool.tile([128, CH], BF, name=f"wq{h}")
                    eng = nc.sync if h % 2 == 0 else nc.gpsimd
                    eng.dma_start(out=wqh[:], in_=wq[h * 128:(h + 1) * 128, :])
                    wq_t.append(wqh)
                for cchunk in range(2):
                    bqc = wpool.tile([128, 1], F32, name=f"bq{cchunk}")
                    nc.scalar.dma_start(
                        out=bqc[:], in_=bq[cchunk * 128:(cchunk + 1) * 128, :])
                    bq_t.append(bqc)
                for cchunk in range(2):
                    woc = wpool.tile([128, HID], BF, name=f"wo{cchunk}")
                    nc.scalar.dma_start(
                        out=woc[:], in_=wo[cchunk * 128:(cchunk + 1) * 128, :])
                    wo_t.append(woc)

            ident = wpool.tile([128, 128], BF, name="ident")
            make_identity(nc, ident[:])

            # persistent v_aug tiles; ones column written once
            va_tiles = [
                [vaug_pool.tile([128, D + 1], BF, tag="vaug", name=f"va{b}_{kc}")
                 for kc in range(NKC)]
                for b in range(B)
            ]
            for b in range(B):
                for kc in range(NKC):
                    nc.gpsimd.memset(va_tiles[b][kc][:, D:D + 1], 1.0)

            # per-batch activation tiles (tags give b0/b1 separate slots)
            qp = {}
            kT2 = {}
            vT = {}
            oT = {}
            for b in range(B):
                qp[b] = [acts.tile([128, S], BF, tag=f"qp{i}", name=f"qp{i}_{b}")
                         for i in range(2)]
                kT2[b] = acts.tile([128, S], BF, tag="kT2", name=f"kT2{b}")
                vT[b] = acts.tile([D, S], BF, tag="vT", name=f"vT{b}")
                oT[b] = [acts.tile([128, S], BF, tag=f"oT{i}", name=f"oT{i}_{b}")
                         for i in range(2)]

            # ---- deferred softmax epilogue ----
            # (un, zs, row, qc, odst, b) -> recip at kc==2, bc+STT at kc==8
            pending = []
            pending_bc = []
            stt_done = {}      # (b, qc) -> count of normalized (rpair, half)
            on_qc_done = {}    # (b, qc) -> callback when all 4 STTs emitted

            def flush_recip():
                while pending:
                    un, zs, row, qc, odst, b = pending.pop(0)
                    key = f"{b}{row}{qc}{len(pending)}"
                    rt = small.tile([1, 512], F32, tag="rt", name=f"rt{key}")
                    nc.vector.reciprocal_approx_fast(rt[:], zs[:])
                    rth = small.tile([1, 512], BF, tag="rth", name=f"rth{key}")
                    nc.vector.tensor_copy(rth[:], rt[:])
                    pending_bc.append((un, rth, row, qc, odst, b))

            def flush_bc():
                while pending_bc:
                    un, rth, row, qc, odst, b = pending_bc.pop(0)
                    qsl = slice(qc * 512, (qc + 1) * 512)
                    key = f"{b}{row}{qc}{len(pending_bc)}"
                    # broadcast 1/Z across the 64 head dims on gpsimd (keeps
                    # the PE free of broadcast matmuls and their LDW stalls)
                    bcs = small.tile([D, 512], BF, tag="bcs", name=f"bcs{key}")
                    nc.gpsimd.partition_broadcast(bcs[:], rth[:], channels=D)
                    nc.vector.tensor_mul(odst[row:row + D, qsl], un[0:D, :], bcs[:])
                    k2 = (b, qc)
                    stt_done[k2] = stt_done.get(k2, 0) + 1
                    if stt_done[k2] == 4 and k2 in on_qc_done:
                        on_qc_done.pop(k2)()

            def flush_epilogue():
                flush_recip()
                flush_bc()

            # ---- work quanta ----
            filler = []        # list of 0-arg closures, FIFO

            def pump(n=1):
                for _ in range(n):
                    if filler:
                        filler.pop(0)()

            def pump_all():
                while filler:
                    filler.pop(0)()

            # projection quanta for one (b, st); act_bias=True routes the
            # PSUM evacuation + bias through ACT (ramp), else DVE.
            def proj_quanta(b, st):
                ssl = slice(st * 512, (st + 1) * 512)
                xt = [None] * NH
                et = [None] * NH
                state = {}

                def e_dma():
                    for h in range(NH):
                        e = es_pool.tile([128, 512], BF, tag="es",
                                         name=f"es{b}{st}{h}")
                        # ramp st0: split enc loads across two queues so the
                        # first kv matmuls aren't DMA-issue-paced
                        eng = (nc.gpsimd if (b == 0 and st == 0 and h % 2)
                               else nc.sync)
                        eng.dma_start(
                            out=e[:], in_=encT[b, h * 128:(h + 1) * 128, ssl])
                        et[h] = e

                def x_dma4(g):
                    # b1 x-loads go on sync: the gpsimd queue must stay
                    # responsive for the 1/Z partition broadcasts
                    def f():
                        if g >= 4:
                            return
                        eng = nc.gpsimd if b == 0 else nc.sync
                        for h in range(4 * g, 4 * g + 4):
                            x = xs_pool.tile([128, 512], BF, tag="xs",
                                             name=f"xs{b}{st}{h}")
                            eng.dma_start(
                                out=x[:], in_=xT[b, h * 128:(h + 1) * 128, ssl])
                            xt[h] = x
                    return f

                def q_kv(i):
                    def f():
                        if i == 0:
                            state["kvp"] = ps_aux.tile(
                                [128, 512], F32, tag="aux", name=f"kvp{b}{st}")
                        kvp = state["kvp"]
                        for h in range(4 * i, 4 * i + 4):
                            nc.tensor.matmul(
                                kvp[:], wkv_t[h][:], et[h][:],
                                start=(h == 0), stop=(h == NH - 1))
                    return f

                def q_kv_evac():
                    kvp = state["kvp"]
                    nc.vector.tensor_scalar_add(
                        kT2[b][0:D, ssl], kvp[0:D, :], bkk_t[0:D, :])
                    nc.vector.tensor_scalar_add(
                        kT2[b][D:128, ssl], kvp[0:D, :], bkk_t[0:D, :])
                    nc.vector.tensor_scalar_add(
                        vT[b][:, ssl], kvp[D:128, :], bvv_t[D:128, :])

                def q_vaug():
                    for j in range(4):
                        kc = st * 4 + j
                        vtp = ps_aux.tile([128, D], BF, tag="aux",
                                          name=f"vtp{b}{kc}")
                        nc.tensor.transpose(
                            vtp[:], vT[b][:, kc * 128:(kc + 1) * 128],
                            ident[0:D, 0:D])
                        nc.vector.tensor_copy(va_tiles[b][kc][:, 0:D], vtp[:])

                def q_q(half, i):
                    def f():
                        key = f"qp{half}"
                        if i == 0:
                            state[key] = ps_aux.tile(
                                [128, 512], F32, tag="aux",
                                name=f"qps{b}{st}{half}")
                        qps = state[key]
                        csl = slice(half * 128, (half + 1) * 128)
                        for h in range(4 * i, 4 * i + 4):
                            nc.tensor.matmul(
                                qps[:], wq_t[h][:, csl], xt[h][:],
                                start=(h == 0), stop=(h == NH - 1))
                    return f

                def q_q_evac(half):
                    def f():
                        qps = state[f"qp{half}"]
                        nc.vector.tensor_scalar_add(
                            qp[b][half][:, ssl], qps[:], bq_t[half][:])
                    return f

                kv_chain = [e_dma] + [q_kv(i) for i in range(4)] + [q_kv_evac, q_vaug]
                # x-tiles arrive in groups of 4, two quanta ahead of use
                qa_chain = [x_dma4(0), x_dma4(1)]
                for i in range(4):
                    qa_chain += [q_q(0, i), x_dma4(i + 2)]
                qa_chain.append(q_q_evac(0))
                qb_chain = [q_q(1, i) for i in range(4)] + [q_q_evac(1)]
                return kv_chain, qa_chain, qb_chain

            # o-projection quanta for (b, qc): 4 s-chunks x 4 hid quarters,
            # one aux slot per quantum so the slot turns over quickly
            def oproj_quanta(b, qc):
                quanta = []
                for si in range(4):
                    s128 = slice((qc * 4 + si) * 128, (qc * 4 + si + 1) * 128)
                    for hh in range(4):
                        def f(s128=s128, hh=hh, si=si):
                            ops = ps_aux.tile([128, 512], F32, tag="aux",
                                              name=f"op{b}{qc}{si}{hh}")
                            hsl = slice(hh * 512, (hh + 1) * 512)
                            nc.tensor.matmul(ops[:], oT[b][0][:, s128],
                                             wo_t[0][:, hsl], start=True, stop=False)
                            nc.tensor.matmul(ops[:], oT[b][1][:, s128],
                                             wo_t[1][:, hsl], start=False, stop=True)
                            osb = osb_pool.tile([128, 512], BF, tag="osb",
                                                name=f"ob{b}{qc}{si}{hh}")
                            nc.vector.tensor_copy(osb[:], ops[:])
                            nc.sync.dma_start(
                                out=out[b, s128, hsl], in_=osb[:])
                        quanta.append(f)
                return quanta

            # attention unit (b, rpair, qc); tick(kc) emits one kc step,
            # pump_rate fillers pulled per odd kc
            class Unit:
                def __init__(self, b, rpair, qc, pump_rate=1):
                    self.b, self.rpair, self.qc = b, rpair, qc
                    self.pump_rate = pump_rate
                    self.qpt = qp[b][rpair]
                    self.odst = oT[b][rpair]
                    self.qsl = slice(qc * 512, (qc + 1) * 512)
                    self.ava = ps_pav.tile([D + 1, 512], F32, tag="pav",
                                           name=f"av{b}{rpair}{qc}a")
                    self.avb = ps_pav.tile([D + 1, 512], F32, tag="pav",
                                           name=f"av{b}{rpair}{qc}b")
                    self.es = [None] * NKC

                def _score(self, kc):
                    b, rpair, qc = self.b, self.rpair, self.qc
                    ksl = slice(kc * 128, (kc + 1) * 128)
                    sct2 = ps_big.tile([128, 1024], F32, tag="big",
                                       name=f"sc{b}{rpair}{qc}{kc}")
                    nc.tensor.matmul(
                        sct2[:, 0:512], kT2[b][0:D, ksl],
                        self.qpt[0:D, self.qsl], start=True, stop=True)
                    nc.tensor.matmul(
                        sct2[:, 512:1024], kT2[b][D:128, ksl],
                        self.qpt[D:128, self.qsl], start=True, stop=True)
                    e2 = epool.tile([128, 1024], BF, tag="e",
                                    name=f"e{b}{rpair}{qc}{kc}")
                    nc.scalar.activation(e2[:], sct2[:], EXP, scale=float(SCALE))
                    self.es[kc] = e2

                def _av(self, kc, stop=False):
                    b = self.b
                    pe2 = self.es[kc]
                    nc.tensor.matmul(
                        self.ava[:], va_tiles[b][kc][:], pe2[:, 0:512],
                        start=(kc == 0), stop=stop)
                    nc.tensor.matmul(
                        self.avb[:], va_tiles[b][kc][:], pe2[:, 512:1024],
                        start=(kc == 0), stop=stop)

                def tick2(self, j):
                    # two score pairs back-to-back (amortizes the PE weight-
                    # buffer stall of entering/leaving row-tiled pairs), then
                    # the AV matmuls of the previous pair of kc's
                    self._score(2 * j)
                    self._score(2 * j + 1)
                    if j > 0:
                        self._av(2 * j - 2)
                        self._av(2 * j - 1)
                    if j == 1:
                        flush_recip()
                    elif j == 4:
                        flush_bc()
                    pump(self.pump_rate if j % 2 == 0 else 2 * self.pump_rate)

                def finish(self):
                    b, rpair, qc = self.b, self.rpair, self.qc
                    self._av(NKC - 2)
                    self._av(NKC - 1, stop=True)
                    # drain each accumulator in ONE copy (frees the PSUM bank
                    # fastest); Z rides along as row D, re-cast to f32 for
                    # the reciprocal later
                    for half, av in ((0, self.ava), (1, self.avb)):
                        key = f"{b}{rpair}{qc}{half}"
                        un = small.tile([D + 1, 512], BF, tag="un", bufs=6,
                                        name=f"un{key}")
                        nc.vector.tensor_copy(un[:], av[:])
                        zs = small.tile([1, 512], F32, tag="zs", name=f"zs{key}")
                        nc.vector.tensor_copy(zs[:], un[D:D + 1, :])
                        pending.append((un, zs, half * D, qc, self.odst, b))

            def emit_unit(b, rpair, qc, pump_rate=1):
                u = Unit(b, rpair, qc, pump_rate)
                for j in range(NKC // 2):
                    u.tick2(j)
                u.finish()

            # =========== emission schedule ===========
            # ramp: projection of b0 (ACT bias; ACT idle anyway) with the
            # first attention unit chunked in per-st so exp starts early
            u0 = None
            for st in range(NST):
                kv_c, qa_c, qb_c = proj_quanta(0, st)
                for f in kv_c:
                    f()
                if st == 0:
                    load_late_weights()
                    for f in qa_c + qb_c:
                        f()
                    u0 = Unit(0, 0, 0)
                    u0.tick2(0)
                else:
                    u0.tick2(2 * st - 1)
                    for f in qa_c:
                        f()
                    u0.tick2(2 * st)
                    for f in qb_c:
                        f()
            u0.tick2(NKC // 2 - 1)
            u0.finish()

            # window 1: rest of attn(b0); filler = proj(b1) then oproj(b0)
            b1_chains = [proj_quanta(1, st) for st in range(NST)]
            # kv of all st first (needed by every b1 unit), then qA st0
            for st in range(NST):
                filler.extend(b1_chains[st][0])
            filler.extend(b1_chains[0][1])           # qA st0

            def need_q(half, st):
                # force-drain filler up to and including qX(st) of b1;
                # queue order guarantees every prerequisite is ahead of it
                chain = b1_chains[st][1 + half]
                while chain[-1] in filler:
                    pump(1)

            for qc in range(4):
                on_qc_done[(0, qc)] = (
                    lambda qc=qc: filler.extend(oproj_quanta(0, qc)))
                if qc > 0:
                    emit_unit(0, 0, qc)
                emit_unit(0, 1, qc)
            # remaining b1 q-projections, ordered so each st's x-tiles die
            # (at qB) before the next st's load: qB0, qA1, qB1, qA2, ...
            filler.extend(b1_chains[0][2])
            for st in range(1, NST):
                filler.extend(b1_chains[st][1])
                filler.extend(b1_chains[st][2])

            # window 2: attn(b1) with filler = rest of b1 proj + oproj(b0/b1)
            for qc in range(4):
                on_qc_done[(1, qc)] = (
                    lambda qc=qc: filler.extend(oproj_quanta(1, qc)))
                need_q(0, qc)
                emit_unit(1, 0, qc, pump_rate=1 if qc < 2 else 2)
                need_q(1, qc)
                # drain harder near the end so the tail is short
                emit_unit(1, 1, qc, pump_rate=1 if qc < 2 else 2)

            flush_epilogue()
            pump_all()

    if not nc.is_finalized():
        nc.finalize()
    return nc


_NC = None
_RUN_KWARGS = {}
_LAST_RESULT = None


def _get_nc():
    global _NC
    if _NC is None:
        _NC = _build_nc()
    return _NC


def kernel(x, encoder_output, Wq, bq, Wk, bk, Wv, bv, Wo, bo):
    nc = _get_nc()
    xT = np.ascontiguousarray(np.asarray(x, np.float32).transpose(0, 2, 1)).astype(BF16)
    encT = np.ascontiguousarray(
        np.asarray(encoder_output, np.float32).transpose(0, 2, 1)).astype(BF16)
    Wq = np.asarray(Wq, np.float32)
    Wk = np.asarray(Wk, np.float32)
    Wv = np.asarray(Wv, np.float32)
    Wo = np.asarray(Wo, np.float32)
    bk = np.asarray(bk, np.float32)
    bv = np.asarray(bv, np.float32)
    in_maps = []
    for c in range(NCORES):
        csl = slice(c * CH, (c + 1) * CH)
        gsl = slice(c * D, (c + 1) * D)
        bk_g = bk[gsl].reshape(D, 1)
        bv_g = bv[gsl].reshape(D, 1)
        in_maps.append({
            "xT": xT,
            "encT": encT,
            "wq": np.ascontiguousarray(Wq[:, csl]).astype(BF16),
            "wkv": np.ascontiguousarray(
                np.concatenate([Wk[:, gsl], Wv[:, gsl]], axis=1)).astype(BF16),
            "wo": np.ascontiguousarray(Wo[csl, :]).astype(BF16),
            "bq": np.ascontiguousarray(
                np.asarray(bq, np.float32)[csl].reshape(CH, 1)),
            "bkk": np.ascontiguousarray(np.concatenate([bk_g, bk_g], axis=0)),
            "bvv": np.ascontiguousarray(np.concatenate([bv_g, bv_g], axis=0)),
        })
    res = run_bass_kernel_spmd(nc, in_maps, list(range(NCORES)), **_RUN_KWARGS)
    global _LAST_RESULT
    _LAST_RESULT = res
    total = np.zeros((B, S, HID), np.float32)
    for c in range(NCORES):
        total += res.results[c]["out"].astype(np.float32)
    return total + np.asarray(bo, np.float32)
